# revision 15
# baseline (speedup 1.0000x reference)
"""NonLocalBlock (embedded-gaussian attention) TRN2 kernel, v3.

Shapes (hardcoded): x [8, 256, 64, 64] fp32.
Per batch element b (one NeuronCore each, 8 cores data-parallel):
  theta/phi/g = 1x1 conv projections of x_b [256, 4096] -> [128, 4096]
  f^T[j, i] = sum_c phi[c, j] theta[c, i]        (4096 x 4096 logits)
  soft = softmax over j  (no max subtraction: |f| <= ~91, exp fits fp32)
  y[ci, i] = sum_j soft[j, i] gT[j, ci]          (normalization deferred)
  out = x + W_w @ (y / Z) + (W_w @ g_b + W_b)    (g bias folded via softmax sum=1)

v3 design (from trace analysis of the 210us baseline):
  - ScalarE exp stream is the metronome: 128 ACTIVATEs of [128,1024]
    PSUM->SBUF at ~1.13us each = ~145us floor.  ScalarE does NOTHING else
    (a dummy exp up front pre-loads the ACT table set off the critical
    path); all evictions/biases live on DVE.
  - fp16 everywhere on the f path (x, theta_w/phi_w, th, ph): fp16 has an
    11-bit mantissa = same effective precision as fp32r's tf32 rounding,
    but gets FWL fast weight loads (~27ns vs 107ns per LDWEIGHTS) and
    halves SBUF/DMA.  Matmul rate is 1 col/cycle for fp16 = same as fp32r.
  - ef (exp values, up to ~2e37) needs bf16 for range; y-matmuls pure
    bf16 (gT bf16).  W-projection pure fp16 (ynt fp16, ww fp16).
  - Z accumulation fully on DVE in bf16 (2x perf mode), split into
    even/odd-j accumulators to halve the bf16 rounding walk; tiny PE
    ones-matmuls fold both into PSUM fp32 at quarter end (exact).
  - Head restructured: first exp fires ~6us in (was 52us).  Only
    quarter 0 of the theta/phi projections runs before the pipeline
    starts; everything else (remaining proj quarters, gT projection,
    x+=wbe bias) is injected into early pipeline slots where PE/DVE
    have slack vs the exp rate.
"""

import numpy as np
from ml_dtypes import bfloat16

import concourse.bacc as bacc
import concourse.mybir as mybir
from concourse import tile
from concourse.bass_utils import run_bass_kernel_spmd

F32 = mybir.dt.float32
F16 = mybir.dt.float16
BF16 = mybir.dt.bfloat16
AF = mybir.ActivationFunctionType

B, C, CI = 8, 256, 128
H, Wd = 64, 64
N = H * Wd              # 4096
NQ = 4                  # i-quarters
QW = N // NQ            # 1024
JB = N // 128           # 32 j-blocks

YLAG = 8        # software-pipeline lag of y-matmuls behind f/exp
ZINV_J = 2      # j-slot of next quarter where recip+broadcast of Z runs
MULT_J = YLAG   # after the trailing y-matmuls of the previous quarter
WPROJ_J0 = 11   # j-slot where previous quarter's W-projection chunks start
WPROJ_STEP = 2  # spacing between W-projection chunks (pw pool breathing room)


def build():
    nc = bacc.Bacc("TRN2", target_bir_lowering=False, debug=False, num_devices=8)

    x_d = nc.dram_tensor("x", [C, N], F16, kind="ExternalInput")
    thw_d = nc.dram_tensor("thw_t", [C, CI], F16, kind="ExternalInput")  # theta_w.T
    phw_d = nc.dram_tensor("phw_t", [C, CI], F16, kind="ExternalInput")  # phi_w.T
    gw_d = nc.dram_tensor("gw_t", [C, CI], F16, kind="ExternalInput")    # g_w.T
    ww_d = nc.dram_tensor("ww_t", [CI, C], F16, kind="ExternalInput")    # W_w.T
    # aux columns: 0=theta_b, 1=phi_b, 2=wbe_eff[:128], 3=wbe_eff[128:], 4=ones
    aux_d = nc.dram_tensor("aux", [128, 5], F32, kind="ExternalInput")
    out_d = nc.dram_tensor("out", [C, N], F32, kind="ExternalOutput")

    with tile.TileContext(nc) as tc:
        with (
            tc.tile_pool(name="const", bufs=1) as cpool,
            tc.tile_pool(name="big", bufs=1) as bigpool,
            tc.tile_pool(name="ef", bufs=10) as efpool,
            tc.tile_pool(name="zpool", bufs=2) as zpool,
            tc.tile_pool(name="ypool", bufs=2) as ypool,
            tc.tile_pool(name="opool", bufs=6) as opool,
            tc.tile_pool(name="pf", bufs=2, space="PSUM") as pf,
            tc.tile_pool(name="py", bufs=1, space="PSUM") as py,
            tc.tile_pool(name="pw", bufs=2, space="PSUM") as pw,
        ):
            # ---------------- weight / input loads ----------------
            # table-preload dummy exp off a memset scratch (no DMA dep):
            # walrus hangs the ACT_TABLE_LOAD off the first ScalarE op, so
            # the ~2.7us table load runs during the input DMAs.
            escr_s = cpool.tile([128, 1], F32, tag="escr_s")
            escr = cpool.tile([128, 1], BF16, tag="escr")
            nc.vector.memset(escr_s[:], 1.0)
            nc.scalar.activation(escr[:], escr_s[:], AF.Exp)

            aux = cpool.tile([128, 5], F32, tag="aux")
            thb, phb = aux[:, 0:1], aux[:, 1:2]
            wbe0, wbe1 = aux[:, 2:3], aux[:, 3:4]
            ones_f32 = aux[:, 4:5]
            ones_b16 = cpool.tile([128, 1], BF16, tag="ones")

            thw = cpool.tile([128, 2 * CI], F16, tag="thw")
            phw = cpool.tile([128, 2 * CI], F16, tag="phw")
            gw = cpool.tile([128, 2 * CI], F16, tag="gw")
            x0 = bigpool.tile([128, N], F16, tag="x0")
            x1 = bigpool.tile([128, N], F16, tag="x1")
            xs = (x0, x1)

            # DMA priority order: the head's critical path is
            # x[0:1024] + thw + phw -> proj q0 -> first f-MM -> first exp.
            # x first halves on sync+gpsimd queues; weights on the (idle
            # until first exp) scalar queue.
            nc.sync.dma_start(x0[:, 0:512], x_d[0:128, 0:512])
            nc.gpsimd.dma_start(x1[:, 0:512], x_d[128:256, 0:512])
            for t, dd in ((thw, thw_d), (phw, phw_d)):
                nc.scalar.dma_start(t[:, 0:CI], dd[0:128, :])
                nc.scalar.dma_start(t[:, CI:2 * CI], dd[128:256, :])
            nc.sync.dma_start(x0[:, 512:QW], x_d[0:128, 512:QW])
            nc.gpsimd.dma_start(x1[:, 512:QW], x_d[128:256, 512:QW])
            nc.sync.dma_start(aux[:], aux_d[:])
            nc.vector.tensor_copy(ones_b16[:], ones_f32)
            nc.gpsimd.dma_start(gw[:, 0:CI], gw_d[0:128, :])
            nc.gpsimd.dma_start(gw[:, CI:2 * CI], gw_d[128:256, :])
            for c in range(1, NQ):
                lo = c * QW
                nc.sync.dma_start(x0[:, lo:lo + QW], x_d[0:128, lo:lo + QW])
                nc.gpsimd.dma_start(x1[:, lo:lo + QW], x_d[128:256, lo:lo + QW])
            ww = cpool.tile([CI, C], F16, tag="ww")
            nc.sync.dma_start(ww[:], ww_d[:])

            th_sb = bigpool.tile([128, N], F16, tag="th")
            ph_sb = bigpool.tile([128, N], F16, tag="ph")
            gT_sb = bigpool.tile([128, N], BF16, tag="gT")

            # ---------------- projection emitters ----------------
            def emit_proj_cols(wt, bias_t, dst, lo, w, pname):
                # th/ph projection for columns [lo, lo+w), w <= 512,
                # through the pw psum pool (keeps the pf pool private to
                # the f/exp stream); DVE evicts with bias.
                pp = pw.tile([128, 512], F32, tag="pw",
                             name=f"pj_{pname}_{lo}")
                for k in range(2):
                    nc.tensor.matmul(
                        pp[:, 0:w],
                        wt[:, k * CI:(k + 1) * CI],
                        xs[k][:, lo:lo + w],
                        start=(k == 0), stop=(k == 1),
                    )
                nc.vector.tensor_scalar_add(
                    dst[:, lo:lo + w], pp[:, 0:w], bias_t)

            def emit_gt_block(j):
                # gT projection for one 128-wide j-block (pure fp16 MMs,
                # bf16 eviction); psum borrowed from the pw pool.
                pg = pw.tile([128, 128], F32, tag="pw", name=f"pg_{j}")
                for k in range(2):
                    nc.tensor.matmul(
                        pg[:],
                        xs[k][:, j * 128:(j + 1) * 128],
                        gw[:, k * CI:(k + 1) * CI],
                        start=(k == 0), stop=(k == 1),
                    )
                nc.vector.tensor_copy(gT_sb[:, j * 128:(j + 1) * 128], pg[:])

            def emit_wbe_add(chunk):
                # x += (W_w @ g_b + W_b), per-partition scalar, one
                # [128,1024] chunk of one c-half at a time (8 chunks).
                half, q = divmod(chunk, NQ)
                t, wb = (x0, wbe0) if half == 0 else (x1, wbe1)
                nc.vector.tensor_scalar_add(
                    t[:, q * QW:(q + 1) * QW], t[:, q * QW:(q + 1) * QW], wb)

            # ---------------- head: minimal pre-pipeline proj ----------------
            # just enough for f-j0: th[0:1024] and ph[0:128]
            emit_proj_cols(thw, thb, th_sb, 0, 512, "th")
            emit_proj_cols(thw, thb, th_sb, 512, 512, "th")
            emit_proj_cols(phw, phb, ph_sb, 0, 128, "ph")

            # injection schedules: t-slot -> list of thunks
            inject = {}

            def add_inject(t, fn):
                inject.setdefault(t, []).append(fn)

            # rest of ph quarter 0 (f-j needs ph block j at t=j)
            add_inject(0, lambda: emit_proj_cols(phw, phb, ph_sb, 128, 384, "ph"))
            add_inject(1, lambda: emit_proj_cols(phw, phb, ph_sb, 512, 512, "ph"))
            # gT blocks: 2 per early slot (y-matmul j needs gT_j at t=j+YLAG)
            for j in range(JB):
                add_inject(2 + j // 2, lambda j=j: emit_gt_block(j))
            # remaining ph quarters (f needs ph quarter k from t=8k)
            for k, base in ((1, 3), (2, 10), (3, 18)):
                for s in range(2):
                    add_inject(base + 2 * s, lambda k=k, s=s: emit_proj_cols(
                        phw, phb, ph_sb, k * QW + s * 512, 512, "ph"))
            # remaining th quarters (i-quarter q starts at t=32q)
            for k, base in ((1, 24), (2, 54), (3, 86)):
                for s in range(2):
                    add_inject(base + 2 * s, lambda k=k, s=s: emit_proj_cols(
                        thw, thb, th_sb, k * QW + s * 512, 512, "th"))
            # x += wbe chunks: after ALL projection reads of that x quarter
            # (th-q proj is the last, injected above), before that quarter's
            # W-proj out-adds at t=32(q+1)+WPROJ_J0.
            for q, base in ((0, 30), (1, 34), (2, 60), (3, 92)):
                add_inject(base, lambda q=q: emit_wbe_add(0 * NQ + q))
                add_inject(base + 2, lambda q=q: emit_wbe_add(1 * NQ + q))

            # ---------------- attention loop emitters ----------------
            def emit_wproj_chunk(qq, ynt_q, chunk):
                # chunk = (ob, s2): W-projection + x add + store for quarter qq
                ob, s2 = divmod(chunk, 2)
                lo = qq * QW + s2 * 512
                pwt = pw.tile([128, 512], F32, tag="pw",
                              name=f"pw_{qq}_{chunk}")
                nc.tensor.matmul(
                    pwt[:],
                    ww[:, ob * CI:(ob + 1) * CI],
                    ynt_q[:, s2 * 512:(s2 + 1) * 512],
                    start=True, stop=True,
                )
                ot = opool.tile([128, 512], F32, tag="o",
                                name=f"o_{qq}_{chunk}")
                nc.vector.tensor_add(
                    ot[:], pwt[:], xs[ob][:, lo:lo + 512])
                nc.sync.dma_start(
                    out_d[ob * 128:(ob + 1) * 128, lo:lo + 512], ot[:])

            state = {}    # per-quarter: pyt, zaccE/O, pzt, ynt, zb
            efs = {}
            T = NQ * JB

            def emit_z_close(q):
                # fold both bf16 Z accumulators into PSUM fp32 (exact)
                st = state[q]
                for s in range(2):
                    pzt = pw.tile([1, 512], F32, tag="pw", name=f"pz_{q}_{s}")
                    st["pzt"][s] = pzt
                    nc.tensor.matmul(
                        pzt[:], ones_b16[:],
                        st["zaccE"][:, s * 512:(s + 1) * 512],
                        start=True, stop=False,
                    )
                    nc.tensor.matmul(
                        pzt[:], ones_b16[:],
                        st["zaccO"][:, s * 512:(s + 1) * 512],
                        start=False, stop=True,
                    )

            # last quarter: j >= LQ_PE_J0 go to PE ones-matmuls directly on
            # ef (PSUM fp32 accumulate), the bf16 accumulators fold in at
            # LQ_PE_J0+1, and the group closes on exp_127 alone — shortens
            # the end-of-kernel critical chain by ~1us.
            LQ_PE_J0 = 24

            def emit_z_pe(q, j, ef):
                st = state[q]
                for s in range(2):
                    if j == LQ_PE_J0:
                        st["pzt"][s] = pw.tile(
                            [1, 512], F32, tag="pw", name=f"pz_{q}_{s}")
                    nc.tensor.matmul(
                        st["pzt"][s][:], ones_b16[:],
                        ef[:, s * 512:(s + 1) * 512],
                        start=(j == LQ_PE_J0), stop=(j == JB - 1),
                    )
                if j == LQ_PE_J0 + 1:
                    for s in range(2):
                        for acc in (st["zaccE"], st["zaccO"]):
                            nc.tensor.matmul(
                                st["pzt"][s][:], ones_b16[:],
                                acc[:, s * 512:(s + 1) * 512],
                                start=False, stop=False,
                            )

            def tail_zinv(q):
                st = state[q]
                zi = zpool.tile([1, QW], F32, tag="zi", name=f"zi_{q}")
                zs = zpool.tile([1, QW], F32, tag="zs", name=f"zs_{q}")
                zb = st["zb"] = zpool.tile([128, QW], F32, tag="zb",
                                           name=f"zb_{q}")
                for s in range(2):
                    nc.vector.reciprocal_approx_accurate(
                        zi[:, s * 512:(s + 1) * 512], st["pzt"][s][:],
                        zs[:, s * 512:(s + 1) * 512],
                    )
                    nc.gpsimd.partition_broadcast(
                        zb[:, s * 512:(s + 1) * 512],
                        zi[:, s * 512:(s + 1) * 512])

            def tail_mult(q):
                st = state[q]
                ynt = st["ynt"] = ypool.tile([128, QW], F16, tag="ynt",
                                             name=f"ynt_{q}")
                nc.vector.tensor_mul(ynt[:], st["pyt"][:], st["zb"][:])

            # Flat software pipeline over t = q*JB + j: f/exp/zacc at t,
            # y-MMs trail by YLAG (crossing quarter boundaries so PE never
            # drains), per-quarter Z-finalize/normalize and the deferred
            # W-projection are injected at fixed offsets into the next
            # quarter, head work injected into early slots.
            for t in range(T + YLAG):
                q, j = divmod(t, JB)
                if t < T:
                    if j == 0:
                        state[q] = {"pzt": [None, None]}
                        state[q]["zaccE"] = zpool.tile(
                            [128, QW], BF16, tag="zaccE", name=f"zaccE_{q}")
                        state[q]["zaccO"] = zpool.tile(
                            [128, QW], BF16, tag="zaccO", name=f"zaccO_{q}")
                    st = state[q]
                    i0 = q * QW
                    pft = pf.tile([128, QW], F32, tag="pf", name=f"pf_{t}")
                    for s in range(2):
                        nc.tensor.matmul(
                            pft[:, s * 512:(s + 1) * 512],
                            ph_sb[:, j * 128:(j + 1) * 128],
                            th_sb[:, i0 + s * 512:i0 + (s + 1) * 512],
                            start=True, stop=True,
                        )
                    ef = efpool.tile([128, QW], BF16, tag="ef", name=f"ef_{t}")
                    efs[t] = ef
                    nc.scalar.activation(ef[:], pft[:], AF.Exp)
                    # Z accumulation on DVE, bf16 2x mode, even/odd split
                    # (last quarter: late j-blocks go straight to PE)
                    if q == NQ - 1 and j >= LQ_PE_J0:
                        emit_z_pe(q, j, ef)
                    else:
                        zacc = st["zaccE"] if j % 2 == 0 else st["zaccO"]
                        if j < 2:
                            nc.vector.tensor_copy(zacc[:], ef[:])
                        else:
                            nc.vector.tensor_add(zacc[:], zacc[:], ef[:])
                        if j == JB - 1:
                            emit_z_close(q)
                    # previous quarter's deferred work
                    if q > 0 and j == ZINV_J:
                        tail_zinv(q - 1)
                    if q > 0 and j == MULT_J:
                        tail_mult(q - 1)
                    if q > 0 and (j - WPROJ_J0) % WPROJ_STEP == 0 and \
                            0 <= (j - WPROJ_J0) // WPROJ_STEP < 4:
                        emit_wproj_chunk(q - 1, state[q - 1]["ynt"],
                                         (j - WPROJ_J0) // WPROJ_STEP)
                    # head / spread injections
                    for fn in inject.pop(t, ()):
                        fn()
                # trailing y accumulation
                ty = t - YLAG
                if ty >= 0:
                    qy, jy = divmod(ty, JB)
                    if jy == 0:
                        state[qy]["pyt"] = py.tile([128, QW], F32, tag="py",
                                                   name=f"py_{qy}")
                    efy = efs.pop(ty)
                    for s in range(2):
                        nc.tensor.matmul(
                            state[qy]["pyt"][:, s * 512:(s + 1) * 512],
                            gT_sb[:, jy * 128:(jy + 1) * 128],
                            efy[:, s * 512:(s + 1) * 512],
                            start=(jy == 0), stop=(jy == JB - 1),
                        )

            # last quarter's tail: per-half pipelined Z-inverse /
            # normalize / W-projection chain, output DMAs spread across
            # the three DMA-capable queues so transfers overlap.
            st3 = state[NQ - 1]
            zi3 = zpool.tile([1, QW], F32, tag="zi", name="zi_3")
            zs3 = zpool.tile([1, QW], F32, tag="zs", name="zs_3")
            zb3 = zpool.tile([128, QW], F32, tag="zb", name="zb_3")
            ynt3 = ypool.tile([128, QW], F16, tag="ynt", name="ynt_3")
            for s in range(2):
                nc.vector.reciprocal_approx_accurate(
                    zi3[:, s * 512:(s + 1) * 512], st3["pzt"][s][:],
                    zs3[:, s * 512:(s + 1) * 512])
                nc.gpsimd.partition_broadcast(
                    zb3[:, s * 512:(s + 1) * 512],
                    zi3[:, s * 512:(s + 1) * 512])
            dmaq = (nc.sync, nc.gpsimd, nc.scalar, nc.sync)
            for s in range(2):
                nc.vector.tensor_mul(
                    ynt3[:, s * 512:(s + 1) * 512],
                    st3["pyt"][:, s * 512:(s + 1) * 512],
                    zb3[:, s * 512:(s + 1) * 512])
                for ob in range(2):
                    lo = (NQ - 1) * QW + s * 512
                    pwt = pw.tile([128, 512], F32, tag="pw",
                                  name=f"pw_3_{ob}_{s}")
                    nc.tensor.matmul(
                        pwt[:],
                        ww[:, ob * CI:(ob + 1) * CI],
                        ynt3[:, s * 512:(s + 1) * 512],
                        start=True, stop=True,
                    )
                    ot = opool.tile([128, 512], F32, tag="o",
                                    name=f"o_3_{ob}_{s}")
                    nc.vector.tensor_add(
                        ot[:], pwt[:], xs[ob][:, lo:lo + 512])
                    dmaq[s * 2 + ob].dma_start(
                        out_d[ob * 128:(ob + 1) * 128, lo:lo + 512], ot[:])

    nc.compile()
    return nc


_CACHE = {}


def _get_nc():
    if "nc" not in _CACHE:
        _CACHE["nc"] = build()
    return _CACHE["nc"]


def _in_maps(x, g_w, g_b, theta_w, theta_b, phi_w, phi_b, W_w, W_b):
    x = np.asarray(x, dtype=np.float32)
    wbe = (np.asarray(W_w, np.float32) @ np.asarray(g_b, np.float32)
           + np.asarray(W_b, np.float32))
    common = {
        "thw_t": np.ascontiguousarray(np.asarray(theta_w, np.float16).T),
        "phw_t": np.ascontiguousarray(np.asarray(phi_w, np.float16).T),
        "gw_t": np.ascontiguousarray(np.asarray(g_w, np.float16).T),
        "ww_t": np.ascontiguousarray(np.asarray(W_w, np.float16).T),
        "aux": np.stack(
            [
                np.asarray(theta_b, np.float32),
                np.asarray(phi_b, np.float32),
                wbe[:128],
                wbe[128:],
                np.ones(128, np.float32),
            ],
            axis=1,
        ),
    }
    return [
        {"x": np.ascontiguousarray(x[b].reshape(C, N).astype(np.float16)),
         **common}
        for b in range(B)
    ]


def run(in_maps, **kw):
    nc = _get_nc()
    return run_bass_kernel_spmd(nc, in_maps, list(range(B)), **kw)


def kernel(**inputs):
    res = run(_in_maps(**inputs))
    out = np.stack([res.results[b]["out"] for b in range(B)])
    return out.reshape(B, C, H, Wd)
